# revision 1
# baseline (speedup 1.0000x reference)
"""Trainium2 Bass kernel for nn_CrossAttention_56092272886201.

Talking-heads cross-attention, b=2, n=m=2048, dim=64, heads=8, dh=dv=8.
Sharding: 8 cores = (batch 2) x (query-chunks of 512). Each core is fully
independent (talking-heads mixes the heads axis, which stays on-core; the
query axis i is sharded), so no collectives are needed.

Per-core layout (i-tile = 16 queries x 8 heads = 128 partitions):
  S[(h,i), j]   = QK^T via block-diagonal packed q (one matmul, K=64)
  E = exp(S)    ACT, fused row-sum -> softmax denominator
  TW            = WD * (1/denom) per partition  (denom + talking-heads fold)
  A^T[j,(g,i)]  = talk matmul, lhsT=E-chunk (output already j-partitioned)
  m2            = sum_g A^2 (squares split ACT/DVE, strided tree adds on GPSIMD)
  r             = exp(-0.5*ln(m2/8+eps))   (rsqrt; ln+exp share one ACT table set)
  P = A*r       broadcast multiply
  out           = P @ V_ln (gamma_t folded into V, beta_t via K=1 matmul)
W_talk is centered host-side over g so the heads-LayerNorm mean is exactly 0.
"""

import numpy as np

DIM = 64
HEADS = 8
N = 2048
B = 2
NCORES = 8
ICHUNK = 512          # queries per core
NT = 32               # i-tiles per core (16 queries each)
EPS = 1e-5

_CACHE = {}
# bf16 stationary/moving operands for the talk and AV matmul chains (QK stays
# fp32, all PSUM accumulation and softmax/LN statistics stay fp32). fp32
# matmuls double-pump on TRN2 (2 HW ops + 2x LDWEIGHTS); bf16 halves the
# PE-array occupancy of the 33-matmul-per-tile talk/AV chains.
USE_BF16 = True


def _build(use_beta, use_gamma):
    import concourse.bacc as bacc
    import concourse.tile as tile
    from concourse import mybir

    F32 = mybir.dt.float32
    F32R = mybir.dt.float32r
    I32 = mybir.dt.int32
    MMDT = mybir.dt.bfloat16 if USE_BF16 else mybir.dt.float32
    AX = mybir.AxisListType.X
    OP = mybir.AluOpType
    AF = mybir.ActivationFunctionType
    RSQRT_MAGIC = 0x5f375a86

    nc = bacc.Bacc()
    d_xT = nc.declare_dram_parameter("xT", [64, ICHUNK], F32R, isOutput=False)
    d_ctxT = nc.declare_dram_parameter("ctxT", [64, N], F32R, isOutput=False)
    d_wqT = nc.declare_dram_parameter("wqT", [64, 64], F32R, isOutput=False)
    d_wkT = nc.declare_dram_parameter("wkT", [64, 64], F32R, isOutput=False)
    d_wvT = nc.declare_dram_parameter("wvT", [64, 64], F32R, isOutput=False)
    d_bdz = nc.declare_dram_parameter("bdz", [1, NT * 128], F32R, isOutput=False)
    d_WD = nc.declare_dram_parameter("WD", [128, 128], F32, isOutput=False)
    d_beta = nc.declare_dram_parameter("beta", [1, 128], F32, isOutput=False)
    d_gtf = nc.declare_dram_parameter("gtf", [1, 64], F32, isOutput=False)
    d_gvf = nc.declare_dram_parameter("gvf", [1, 64], F32, isOutput=False)
    d_bvf = nc.declare_dram_parameter("bvf", [1, 64], F32, isOutput=False)
    d_mask = nc.declare_dram_parameter("mask", [1, 64], F32, isOutput=False)
    d_ones = nc.declare_dram_parameter("ones", [128, 1], F32, isOutput=False)
    d_out = nc.declare_dram_parameter("out", [NT * 128, 8], F32, isOutput=True)

    import concourse.bass as bass

    def bcast_ap(ap, levels):
        return bass.AP(tensor=ap.tensor, offset=ap.offset, ap=levels)

    with tile.TileContext(nc) as tc:
        with tc.tile_pool(name="statics", bufs=1) as st:
            xT = st.tile([64, ICHUNK], F32R)
            ctxT = st.tile([64, N], F32R)
            wqT = st.tile([64, 64], F32R)
            wkT = st.tile([64, 64], F32R)
            wvT = st.tile([64, 64], F32R)
            WD = st.tile([128, 128], F32)
            beta = st.tile([1, 128], F32)
            gtf = st.tile([128, 64], F32)
            gvf = st.tile([128, 64], F32)
            bvf = st.tile([128, 64], F32)
            # head-select mask replicated to all partitions via DMA broadcast
            mask = st.tile([128, 64], F32)
            ones = st.tile([128, 1], F32)
            for sb, dr in ((xT, d_xT), (ctxT, d_ctxT), (wqT, d_wqT),
                           (wkT, d_wkT), (wvT, d_wvT), (WD, d_WD),
                           (beta, d_beta), (ones, d_ones)):
                nc.sync.dma_start(out=sb[:], in_=dr[:])
            # replicate [1,64] host rows across all 128 partitions
            for sb, dr in ((gtf, d_gtf), (gvf, d_gvf), (bvf, d_bvf)):
                nc.sync.dma_start(
                    out=sb[:], in_=bcast_ap(dr[:], [[0, 128], [1, 64]]))
            # mask[(g,i),(h,d)] = (h==g): zero then fill 8 diagonal blocks
            # from the [1,64] host ones-row via broadcast DMAs.
            mrow = d_mask.rearrange("o (h d) -> o h d", h=8)
            nc.vector.memset(mask[:], 0.0)

            qT = st.tile([64, ICHUNK], F32R)
            kT = st.tile([64, N], F32R)
            # int32 magic row for the fast-inverse-sqrt seed (heads-LN rsqrt
            # computed on DVE; keeps the ACT engine on one table set)
            magic = st.tile([128, 1], I32)
            nc.vector.memset(magic[:], RSQRT_MAGIC)
            Vraw = st.tile([128, 1024], F32)
            Vn = st.tile([128, 1024], F32)
            Vng = st.tile([128, 1024], F32)
            BD = st.tile([64, NT * 128], F32R)
            vs_sb = st.tile([1, 1024], F32)
            Vsum = st.tile([1, 64], F32)

            # ---------------- prologue ----------------
            # fp32r matmul operands: bit-identical to fp32 but 1 cycle/row on
            # the PE array (vs 4 for fp32) when the moving dim is >= 256
            with tc.tile_pool(name="pps", bufs=1, space="PSUM") as pps:
                qps = pps.tile([64, ICHUNK], F32, tag="q")
                nc.tensor.matmul(qps[:], wqT[:], xT[:], start=True, stop=True)
                nc.scalar.copy(out=qT[:], in_=qps[:])
                for q4 in range(4):
                    kps = pps.tile([64, 512], F32, tag="k")
                    nc.tensor.matmul(kps[:], wkT[:],
                                     ctxT[:, q4 * 512:(q4 + 1) * 512],
                                     start=True, stop=True)
                    nc.scalar.copy(out=kT[:, q4 * 512:(q4 + 1) * 512], in_=kps[:])
                for c in range(16):
                    vps = pps.tile([128, 64], F32, tag="v")
                    nc.tensor.matmul(vps[:], ctxT[:, c * 128:(c + 1) * 128],
                                     wvT[:], start=True, stop=True)
                    nc.vector.tensor_copy(out=Vraw[:, c * 64:(c + 1) * 64],
                                          in_=vps[:])

                # per-head LayerNorm of v over d (groups of 8 in free dim)
                MU8 = st.tile([128, 128], F32)
                S2 = st.tile([128, 128], F32)
                Vsq = st.tile([128, 1024], F32)
                v4 = Vraw[:].rearrange("p (c h d) -> p c h d", h=8, d=8)
                nc.vector.tensor_reduce(out=MU8[:], in_=v4, axis=AX, op=OP.add)
                nc.vector.tensor_mul(out=Vsq[:], in0=Vraw[:], in1=Vraw[:])
                nc.vector.tensor_reduce(
                    out=S2[:], in_=Vsq[:].rearrange("p (c h d) -> p c h d", h=8, d=8),
                    axis=AX, op=OP.add)
                mu = st.tile([128, 128], F32)
                nc.vector.tensor_scalar_mul(out=mu[:], in0=MU8[:], scalar1=0.125)
                musq = st.tile([128, 128], F32)
                nc.vector.tensor_mul(out=musq[:], in0=mu[:], in1=mu[:])
                varv = st.tile([128, 128], F32)
                nc.vector.tensor_scalar_mul(out=varv[:], in0=S2[:],
                                            scalar1=0.125)
                nc.vector.tensor_sub(out=varv[:], in0=varv[:], in1=musq[:])
                nc.vector.tensor_scalar_add(out=varv[:], in0=varv[:],
                                            scalar1=float(EPS))
                lnv = st.tile([128, 128], F32)
                nc.scalar.activation(out=lnv[:], in_=varv[:], func=AF.Ln)
                rv = st.tile([128, 128], F32)
                nc.scalar.activation(out=rv[:], in_=lnv[:], func=AF.Exp,
                                     scale=-0.5)
                muv = mu[:].rearrange("p (c h) -> p c h", h=8)
                mub = bcast_ap(muv, [muv.ap[0], muv.ap[1], muv.ap[2], [0, 8]])
                rvv = rv[:].rearrange("p (c h) -> p c h", h=8)
                rvb = bcast_ap(rvv, [rvv.ap[0], rvv.ap[1], rvv.ap[2], [0, 8]])
                nc.vector.tensor_sub(out=v4, in0=v4, in1=mub)
                nc.vector.tensor_mul(out=v4, in0=v4, in1=rvb)
                v3 = Vraw[:].rearrange("p (c hd) -> p c hd", hd=64)
                gva = gvf[:]
                gvb = bcast_ap(gva, [gva.ap[0], [0, 16], [1, 64]])
                bva = bvf[:]
                bvb = bcast_ap(bva, [bva.ap[0], [0, 16], [1, 64]])
                nc.vector.tensor_mul(out=Vn[:].rearrange("p (c hd) -> p c hd", hd=64),
                                     in0=v3, in1=gvb)
                nc.vector.tensor_add(out=Vn[:].rearrange("p (c hd) -> p c hd", hd=64),
                                     in0=Vn[:].rearrange("p (c hd) -> p c hd", hd=64),
                                     in1=bvb)
                if use_beta:
                    for hf in range(2):
                        vsps = pps.tile([1, 512], F32, tag="vs")
                        nc.tensor.matmul(vsps[:], ones[:],
                                         Vn[:, hf * 512:(hf + 1) * 512],
                                         start=True, stop=True)
                        nc.vector.tensor_copy(
                            out=vs_sb[:, hf * 512:(hf + 1) * 512], in_=vsps[:])
                    vsv = vs_sb[:]
                    nc.vector.tensor_reduce(
                        out=Vsum[:],
                        in_=bcast_ap(vsv, [vsv.ap[0], [1, 64], [64, 16]]),
                        axis=AX, op=OP.add)
                if use_gamma:
                    gta = gtf[:]
                    gtb = bcast_ap(gta, [gta.ap[0], [0, 16], [1, 64]])
                    nc.vector.tensor_mul(
                        out=Vng[:].rearrange("p (c hd) -> p c hd", hd=64),
                        in0=Vn[:].rearrange("p (c hd) -> p c hd", hd=64), in1=gtb)
                    AVrhs = Vng
                else:
                    AVrhs = Vn
                Vng16 = st.tile([128, 1024], MMDT)
                nc.vector.tensor_copy(out=Vng16[:], in_=AVrhs[:])
                AVrhs = Vng16

                # rebuild mask properly: zero, then write 8 diagonal blocks
                for g in range(8):
                    nc.sync.dma_start(
                        out=mask[g * 16:(g + 1) * 16, g * 8:(g + 1) * 8],
                        in_=bcast_ap(mrow[:, g, :], [[0, 16], [1, 8]]),
                    )

                # block-diagonal packed q: BD[(h,d), (t, h, i16)] = qT[(h,d), (t,i)]
                # (zero-fill via DMA broadcast: memset can't write f32r tiles)
                nc.sync.dma_start(
                    out=BD[:],
                    in_=bcast_ap(d_bdz[:], [[0, 64], [1, NT * 128]]))
                BD3 = BD[:].rearrange("p (t c) -> p t c", c=128)
                qT3 = qT[:].rearrange("p (t i) -> p t i", i=16)
                for h in range(8):
                    nc.sync.dma_start(
                        out=BD3[h * 8:(h + 1) * 8, :, h * 16:(h + 1) * 16],
                        in_=qT3[h * 8:(h + 1) * 8, :, :])

            # ---------------- main loop ----------------
            with tc.tile_pool(name="sps", bufs=2, space="PSUM") as sps, \
                 tc.tile_pool(name="aps", bufs=3, space="PSUM") as aps, \
                 tc.tile_pool(name="avps", bufs=1, space="PSUM") as avps, \
                 tc.tile_pool(name="le", bufs=2) as le, \
                 tc.tile_pool(name="lsq", bufs=2) as lsq, \
                 tc.tile_pool(name="lar", bufs=2) as lar, \
                 tc.tile_pool(name="lp", bufs=2) as lp, \
                 tc.tile_pool(name="lt", bufs=2) as lt, \
                 tc.tile_pool(name="lsm", bufs=4) as lsm, \
                 tc.tile_pool(name="lout", bufs=3) as lout:
                def front(t):
                    """QK + softmax-exp + denominator + tw for tile t."""
                    bd_t = BD[:, t * 128:(t + 1) * 128]
                    E = le.tile([128, N], MMDT, tag="E", name=f"E{t}")
                    den4 = lsm.tile([128, 2], F32, tag="den4", name=f"d4_{t}")
                    for q2 in range(2):
                        s_t = sps.tile([128, 1024], F32, tag="s",
                                       name=f"s{t}_{q2}")
                        for qh in range(2):
                            nc.tensor.matmul(
                                s_t[:, qh * 512:(qh + 1) * 512], bd_t,
                                kT[:, (q2 * 2 + qh) * 512:(q2 * 2 + qh + 1) * 512],
                                start=True, stop=True)
                        nc.scalar.activation(
                            out=E[:, q2 * 1024:(q2 + 1) * 1024], in_=s_t[:],
                            func=AF.Exp, accum_out=den4[:, q2:q2 + 1])
                    den = lsm.tile([128, 1], F32, tag="den", name=f"dn{t}")
                    nc.vector.tensor_reduce(out=den[:], in_=den4[:], axis=AX,
                                            op=OP.add)
                    rden = lsm.tile([128, 1], F32, tag="rden", name=f"rd{t}")
                    nc.vector.reciprocal_approx_fast(out=rden[:], in_=den[:])
                    tw = lt.tile([128, 128], MMDT, tag="tw", name=f"tw{t}")
                    rda = rden[:]
                    rdb = bcast_ap(rda, [rda.ap[0], [0, 128]])
                    nc.vector.tensor_tensor(out=tw[:], in0=WD[:], in1=rdb,
                                            op=OP.mult)
                    return E, tw

                E, tw = front(0)
                for t in range(NT):
                    # early PSUM evacuation: ACT copies each talk chunk to
                    # bf16 SBUF right away (one PSUM read frees the bank),
                    # DVE squares the SBUF copy, gpsimd starts that chunk's
                    # tree-add. The m2 -> rsqrt -> P chain runs per column
                    # HALF so half A's rsqrt/P/AV overlap half B's squares.
                    Araw = lar.tile([128, N], MMDT, tag="Araw")
                    SQ = lsq.tile([128, N], MMDT, tag="SQ")
                    T64 = lsq.tile([128, 1024], F32, tag="T64")
                    P = lp.tile([128, N], MMDT, tag="P")
                    t64v = T64[:].rearrange("p (c x) -> p c x", x=64)
                    mg = magic[:]
                    mgb = bcast_ap(mg, [mg.ap[0], [0, 128]])
                    for b4 in range(4):
                        a_t = aps.tile([128, 512], F32, tag="a")
                        for cl in range(4):
                            c = b4 * 4 + cl
                            nc.tensor.matmul(a_t[:, cl * 128:(cl + 1) * 128],
                                             E[:, c * 128:(c + 1) * 128], tw[:],
                                             start=True, stop=True)
                        ar_b = Araw[:, b4 * 512:(b4 + 1) * 512]
                        nc.scalar.copy(out=ar_b, in_=a_t[:])
                        sq_b = SQ[:, b4 * 512:(b4 + 1) * 512]
                        nc.vector.tensor_mul(out=sq_b, in0=ar_b, in1=ar_b)
                        sqv = sq_b.rearrange("p (c x) -> p c x", x=128)
                        nc.gpsimd.tensor_add(
                            out=t64v[:, b4 * 4:(b4 + 1) * 4, :],
                            in0=sqv[:, :, 0:64], in1=sqv[:, :, 64:128])
                        if b4 % 2 == 0:
                            continue
                        # finish this half's m2 -> rsqrt -> P chain
                        h = b4 // 2
                        t64h = T64[:, h * 512:(h + 1) * 512].rearrange(
                            "p (c x) -> p c x", x=64)
                        T32h = lsq.tile([128, 256], F32, tag=f"T32{h}")
                        t32v = T32h[:].rearrange("p (c x) -> p c x", x=32)
                        nc.gpsimd.tensor_add(out=t32v, in0=t64h[:, :, 0:32],
                                             in1=t64h[:, :, 32:64])
                        M2h = lsm.tile([128, 128], F32, tag=f"M2{h}")
                        m2v = M2h[:].rearrange("p (c x) -> p c x", x=16)
                        nc.vector.tensor_tensor(
                            out=m2v, in0=t32v[:, :, 0:16],
                            in1=t32v[:, :, 16:32], op=OP.add)
                        VPh = lsm.tile([128, 128], F32, tag=f"VP{h}")
                        nc.vector.tensor_scalar(
                            out=VPh[:], in0=M2h[:], scalar1=0.125,
                            scalar2=float(EPS), op0=OP.mult, op1=OP.add)
                        # rsqrt via fast-inverse-sqrt seed + one Newton step
                        # (no ACT tables: scalar engine stays on the exp set)
                        Y0h = lsm.tile([128, 128], F32, tag=f"Y0{h}")
                        SHh = lsm.tile([128, 128], I32, tag=f"SH{h}")
                        nc.vector.tensor_scalar(
                            out=SHh[:], in0=VPh.bitcast(I32)[:], scalar1=1,
                            scalar2=None, op0=OP.logical_shift_right)
                        nc.gpsimd.tensor_tensor(
                            out=Y0h.bitcast(I32)[:], in0=mgb, in1=SHh[:],
                            op=OP.subtract)
                        YSQh = lsm.tile([128, 128], F32, tag=f"YSQ{h}")
                        nc.gpsimd.tensor_mul(out=YSQh[:], in0=Y0h[:],
                                             in1=Y0h[:])
                        TNh = lsm.tile([128, 128], F32, tag=f"TN{h}")
                        nc.vector.scalar_tensor_tensor(
                            out=TNh[:], in0=VPh[:], scalar=-0.5, in1=YSQh[:],
                            op0=OP.mult, op1=OP.mult)
                        Rbh = lsm.tile([128, 128], MMDT, tag=f"Rb{h}")
                        nc.vector.scalar_tensor_tensor(
                            out=Rbh[:], in0=TNh[:], scalar=1.5, in1=Y0h[:],
                            op0=OP.add, op1=OP.mult)
                        Rv = Rbh[:].rearrange("p (c i) -> p c i", i=16)
                        rb = bcast_ap(Rv, [Rv.ap[0], Rv.ap[1], [0, 8],
                                           Rv.ap[2]])
                        av_in = Araw[:, h * 1024:(h + 1) * 1024].rearrange(
                            "p (c g i) -> p c g i", g=8, i=16)
                        pv = P[:, h * 1024:(h + 1) * 1024].rearrange(
                            "p (c g i) -> p c g i", g=8, i=16)
                        nc.vector.tensor_mul(out=pv, in0=av_in, in1=rb)
                    av = avps.tile([128, 64], F32, tag="av")
                    for c in range(16):
                        nc.tensor.matmul(av[:], P[:, c * 128:(c + 1) * 128],
                                         AVrhs[:, c * 64:(c + 1) * 64],
                                         start=(c == 0),
                                         stop=(c == 15 and not use_beta))
                    if use_beta:
                        nc.tensor.matmul(av[:], beta[:], Vsum[:],
                                         start=False, stop=True)
                    EX = lout.tile([128, 64], F32, tag="EX")
                    nc.vector.tensor_mul(out=EX[:], in0=av[:], in1=mask[:])
                    RES = lout.tile([128, 8], F32, tag="RES")
                    nc.vector.tensor_reduce(
                        out=RES[:],
                        in_=EX[:].rearrange("p (h d) -> p d h", h=8),
                        axis=AX, op=OP.add)
                    nc.sync.dma_start(out=d_out[t * 128:(t + 1) * 128, :],
                                      in_=RES[:])
                    if t + 1 < NT:
                        E, tw = front(t + 1)
    nc.compile()
    return nc


def _get_module(use_beta, use_gamma):
    key = (use_beta, use_gamma)
    if key not in _CACHE:
        _CACHE[key] = _build(use_beta, use_gamma)
    return _CACHE[key]


def kernel(x, context, Wq, Wkv, g_v, b_v, W_talk, g_t, b_t, **_unused):
    from concourse.bass_utils import run_bass_kernel_spmd

    x = np.asarray(x, np.float32)
    context = np.asarray(context, np.float32)
    Wq = np.asarray(Wq, np.float32)
    Wkv = np.asarray(Wkv, np.float32)
    g_v = np.asarray(g_v, np.float32)
    b_v = np.asarray(b_v, np.float32)
    W_talk = np.asarray(W_talk, np.float32)
    g_t = np.asarray(g_t, np.float32)
    b_t = np.asarray(b_t, np.float32)

    use_beta = bool(np.any(b_t != 0.0))
    use_gamma = bool(np.any(g_t != 1.0))
    nc = _get_module(use_beta, use_gamma)

    wqT = np.ascontiguousarray(Wq.T) * np.float32(DIM ** -0.5)
    wkT = np.ascontiguousarray(Wkv[:DIM, :].T)
    wvT = np.ascontiguousarray(Wkv[DIM:, :].T)
    Wc = W_talk - W_talk.mean(axis=0, keepdims=True)
    WD = np.zeros((8, 16, 8, 16), np.float32)
    for i in range(16):
        WD[:, i, :, i] = Wc.T          # WD[h,i,g,i] = Wc[g,h]
    WD = np.ascontiguousarray(WD.reshape(128, 128))
    beta = np.ascontiguousarray(np.repeat(b_t, 16)[None, :])
    gtf = np.ascontiguousarray(np.repeat(g_t, 8)[None, :])
    gvf = np.ascontiguousarray(np.tile(g_v, 8)[None, :])
    bvf = np.ascontiguousarray(np.tile(b_v, 8)[None, :])
    mrow = np.ones((1, 64), np.float32)
    ones = np.ones((128, 1), np.float32)
    bdz = np.zeros((1, NT * 128), np.float32)

    in_maps = []
    for c in range(NCORES):
        b = c // 4
        i0 = (c % 4) * ICHUNK
        in_maps.append({
            "xT": np.ascontiguousarray(x[b, i0:i0 + ICHUNK, :].T),
            "ctxT": np.ascontiguousarray(context[b].T),
            "wqT": wqT, "wkT": wkT, "wvT": wvT, "WD": WD, "beta": beta,
            "gtf": gtf, "gvf": gvf, "bvf": bvf, "mask": mrow, "ones": ones,
            "bdz": bdz,
        })
    trace_dir = globals().get("TRACE_TMPDIR")
    if trace_dir:
        res = run_bass_kernel_spmd(nc, in_maps, list(range(NCORES)),
                                   trace=True, tmpdir=trace_dir)
        globals()["LAST_EXEC_NS"] = res.exec_time_ns
    else:
        res = run_bass_kernel_spmd(nc, in_maps, list(range(NCORES)))
    out = np.empty((B, 2048, DIM), np.float32)
    for c in range(NCORES):
        b = c // 4
        i0 = (c % 4) * ICHUNK
        o = res.results[c]["out"].reshape(NT, 8, 16, 8)
        out[b, i0:i0 + ICHUNK, :] = (
            o.transpose(0, 2, 1, 3).reshape(ICHUNK, DIM))
    return out



# revision 3
# speedup vs baseline: 1.2861x; 1.2861x over previous
"""Trainium2 Bass kernel for nn_CrossAttention_56092272886201 (227us).

Talking-heads cross-attention, b=2, n=m=2048, dim=64, heads=8, dh=dv=8.
Sharding: 8 cores = (batch 2) x (query-chunks of 512); each core fully
independent (query axis i is sharded; heads stay on-core), no collectives.

Per-core layout (i-tile = 16 queries x 8 heads = 128 partitions):
  S[(h,i),j] = QK^T via block-diagonal packed q, all-bf16 matmuls
  E = exp(S)  ACT, fused row-sum -> softmax denominator
  tw          = WD * rden via tensor_scalar with per-partition AP scalar
  A^T[j,(g,i)] talk matmul (lhsT=E chunks), PSUM evac 3xACT copy + 1xDVE
  T1          = A[g]^2+A[g+4]^2 via CUSTOM DVE op (square+pairsum fused)
  vp          = grouped sum + eps (tree L2 + stt, eps folded)
  r           = rsqrt via magic-constant seed (DVE shift, GPSIMD sub)
                + CUSTOM DVE fused Newton step y*((y^2*v)*-0.5+1.5)
  P = A*r     broadcast multiply (2x bf16), AV matmuls interleaved
  out         = P @ Vn (sqrt8*g_t*g_v folded into V-layernorm scale),
                mask-select h==g, reduce, DMA out
vs the 350us baseline: bf16 QK (was fp32-HIGH 4x), custom fused DVE ops
(~1.3us/tile), GPSIMD strided tree adds eliminated, single ACT table set,
host-packed constants.
"""

import numpy as np

DIM = 64
HEADS = 8
N = 2048
B = 2
NCORES = 8
ICHUNK = 512          # queries per core
NT = 32               # i-tiles per core (16 queries each)
EPS = 1e-5

_CACHE = {}

_CUSTOM = {}


def _register_custom_ops():
    """Register fused DVE micro-op programs (square+pairsum, Newton rsqrt)."""
    if _CUSTOM:
        return _CUSTOM
    import numpy as np
    from concourse import dve_ops
    from concourse.dve_spec import Spec, Src0, Src1, C0, C1, sq, lower
    from concourse.dve_spec import _has_src1
    from concourse.dve_uop import DveOpSpec

    def mk(name, spec):
        if name in dve_ops._SUB_OPCODE_FOR_NAME:
            return next(o for o in dve_ops.OPS if o.name == name)
        row = dve_ops._CUSTOM_DVE_ROW_BASE + len(dve_ops.OPS)
        assert row < 0x20
        dve_ops._SUB_OPCODE_FOR_NAME[name] = row
        shas = {}
        for ver in ("v3", "v4"):
            uops = lower(spec, ver=ver)
            shas[ver] = DveOpSpec(name=name, opcode=row, uops=uops,
                                  rd1_en=_has_src1(spec)).sha(ver)
        op = dve_ops.DveOp(name, spec, subdim=False, uops_sha=shas)
        dve_ops.OPS.append(op)
        dve_ops.CUSTOM_DVE_SPECS[name] = op.spec
        return op

    # out = in0^2 + in1^2   (squares + first tree level in one pass)
    _CUSTOM["sqsum2"] = mk("SQSUM2_ANT", Spec(
        body=sq(Src0) + sq(Src1),
        reference=lambda in0, in1, s0, s1, imm2: (
            in0.astype(np.float32) ** 2 + in1.astype(np.float32) ** 2),
    ))
    # out = y*((y^2*v)*s0 + s1)  -- one Newton step of rsqrt
    _CUSTOM["newton"] = mk("NEWTON_RSQ_ANT", Spec(
        body=Src0 * ((sq(Src0) * Src1) * C0 + C1),
        reference=lambda in0, in1, s0, s1, imm2: (
            in0.astype(np.float32)
            * ((in0.astype(np.float32) ** 2 * in1) * s0 + s1)),
    ))
    return _CUSTOM




def _build(use_beta):
    cust = _register_custom_ops()
    import concourse.bacc as bacc
    import concourse.tile as tile
    from concourse import mybir

    F32 = mybir.dt.float32
    BF16 = mybir.dt.bfloat16
    I32 = mybir.dt.int32
    AX = mybir.AxisListType.X
    OP = mybir.AluOpType
    AF = mybir.ActivationFunctionType
    RSQRT_MAGIC = 0x5f375a86

    nc = bacc.Bacc()
    d_xT = nc.declare_dram_parameter("xT", [64, ICHUNK], BF16, isOutput=False)
    d_ctxT = nc.declare_dram_parameter("ctxT", [64, N], BF16, isOutput=False)
    # wq | wk | wv packed [64, 192]
    d_wp = nc.declare_dram_parameter("wp", [64, 192], BF16, isOutput=False)
    # WD[128] | mask[64] | vscale[64] | vshift[64]  (f32, rows replicated)
    d_cst = nc.declare_dram_parameter("cst", [128, 320], F32, isOutput=False)
    d_out = nc.declare_dram_parameter("out", [NT * 128, 8], F32, isOutput=True)
    if use_beta:
        d_beta = nc.declare_dram_parameter("beta", [1, 128], BF16,
                                           isOutput=False)
        d_ones = nc.declare_dram_parameter("ones", [128, 1], BF16,
                                           isOutput=False)

    import concourse.bass as bass

    def bcast_ap(ap, levels):
        return bass.AP(tensor=ap.tensor, offset=ap.offset, ap=levels)

    with tile.TileContext(nc) as tc:
        with tc.tile_pool(name="statics", bufs=1) as st:
            xT = st.tile([64, ICHUNK], BF16)
            ctxT = st.tile([64, N], BF16)
            wp = st.tile([64, 192], BF16)
            cst = st.tile([128, 320], F32)
            nc.sync.dma_start(out=xT[:], in_=d_xT[:])
            nc.sync.dma_start(out=ctxT[:], in_=d_ctxT[:])
            nc.sync.dma_start(out=wp[:], in_=d_wp[:])
            nc.sync.dma_start(out=cst[:], in_=d_cst[:])
            WD = cst[:, 0:128]
            mask = cst[:, 128:192]
            vscale = cst[:, 192:256]     # sqrt8 * g_v[d] * g_t[h], [128,(h,d)]
            vshift = cst[:, 256:320]     # sqrt8 * b_v[d] * g_t[h]
            if use_beta:
                beta = st.tile([1, 128], BF16)
                ones = st.tile([128, 1], BF16)
                nc.sync.dma_start(out=beta[:], in_=d_beta[:])
                nc.sync.dma_start(out=ones[:], in_=d_ones[:])

            qT = st.tile([64, ICHUNK], BF16)
            kT = st.tile([64, N], BF16)
            BD = st.tile([64, NT * 128], BF16)
            magic = st.tile([128, 1], I32)
            nc.vector.memset(magic[:], RSQRT_MAGIC)
            Vraw = st.tile([128, 1024], BF16)
            Vn = st.tile([128, 1024], BF16)
            vs_sb = st.tile([1, 1024], BF16)
            Vsum = st.tile([1, 64], BF16)

            # ---------------- prologue ----------------
            nc.vector.memset(BD[:], 0.0)
            with tc.tile_pool(name="pps", bufs=2, space="PSUM") as pps:
                # q projection
                qps = pps.tile([64, ICHUNK], F32, tag="q")
                nc.tensor.matmul(qps[:], wp[:, 0:64], xT[:],
                                 start=True, stop=True)
                nc.scalar.copy(out=qT[:], in_=qps[:])
                # k projection
                for q4 in range(4):
                    kps = pps.tile([64, 512], F32, tag="k")
                    nc.tensor.matmul(kps[:], wp[:, 64:128],
                                     ctxT[:, q4 * 512:(q4 + 1) * 512],
                                     start=True, stop=True)
                    nc.scalar.copy(out=kT[:, q4 * 512:(q4 + 1) * 512],
                                   in_=kps[:])
                # block-diagonal packed q (BD zeroed by memset above)
                BD3 = BD[:].rearrange("p (t c) -> p t c", c=128)
                qT3 = qT[:].rearrange("p (t i) -> p t i", i=16)
                for h in range(8):
                    nc.sync.dma_start(
                        out=BD3[h * 8:(h + 1) * 8, :, h * 16:(h + 1) * 16],
                        in_=qT3[h * 8:(h + 1) * 8, :, :])
                # v projection: Vraw[j, (c,h,d)] bf16
                for c4 in range(4):
                    vps = pps.tile([128, 256], F32, tag="v")
                    for ci in range(4):
                        c = c4 * 4 + ci
                        nc.tensor.matmul(vps[:, ci * 64:(ci + 1) * 64],
                                         ctxT[:, c * 128:(c + 1) * 128],
                                         wp[:, 128:192], start=True, stop=True)
                    nc.vector.tensor_copy(
                        out=Vraw[:, c4 * 256:(c4 + 1) * 256], in_=vps[:])

                # ---- per-head LayerNorm of v over d (groups of 8) ----
                v4 = Vraw[:].rearrange("p (c h d) -> p c h d", h=8, d=8)
                MU8 = st.tile([128, 128], F32)
                nc.vector.tensor_reduce(
                    out=MU8[:].rearrange("p (c h) -> p c h", h=8),
                    in_=v4, axis=AX, op=OP.add)
                Vsq = st.tile([128, 1024], BF16)
                nc.vector.tensor_mul(out=Vsq[:], in0=Vraw[:], in1=Vraw[:])
                s4 = Vsq[:].rearrange("p (c h d) -> p c h d", h=8, d=8)
                S2 = st.tile([128, 128], F32)
                nc.vector.tensor_reduce(
                    out=S2[:].rearrange("p (c h) -> p c h", h=8),
                    in_=s4, axis=AX, op=OP.add)
                # mu = MU8/8 ; vp = S2/8 - mu^2 + eps
                mu = st.tile([128, 128], F32)
                nc.vector.tensor_scalar(out=mu[:], in0=MU8[:], scalar1=0.125,
                                        scalar2=None, op0=OP.mult)
                musq = st.tile([128, 128], F32)
                nc.vector.tensor_tensor(out=musq[:], in0=mu[:], in1=mu[:],
                                        op=OP.mult)
                nc.vector.tensor_scalar(out=musq[:], in0=musq[:],
                                        scalar1=float(EPS), scalar2=None,
                                        op0=OP.subtract)
                vp = st.tile([128, 128], F32)
                nc.vector.scalar_tensor_tensor(
                    out=vp[:], in0=S2[:], scalar=0.125, in1=musq[:],
                    op0=OP.mult, op1=OP.subtract)
                # rv = rsqrt(vp) via magic + 2 Newton steps
                mg = magic[:]
                mgb = bcast_ap(mg, [mg.ap[0], [0, 128]])
                sh = st.tile([128, 128], I32)
                nc.vector.tensor_scalar(out=sh[:], in0=vp.bitcast(I32)[:],
                                        scalar1=1, scalar2=None,
                                        op0=OP.logical_shift_right)
                y0 = st.tile([128, 128], F32)
                nc.gpsimd.tensor_tensor(out=y0.bitcast(I32)[:], in0=mgb,
                                        in1=sh[:], op=OP.subtract)
                yv = y0
                for it in range(1):
                    ysq = st.tile([128, 128], F32, name=f"vln_ysq{it}")
                    nc.vector.tensor_tensor(out=ysq[:], in0=yv[:], in1=yv[:],
                                            op=OP.mult)
                    tn = st.tile([128, 128], F32, name=f"vln_tn{it}")
                    nc.vector.scalar_tensor_tensor(
                        out=tn[:], in0=vp[:], scalar=-0.5, in1=ysq[:],
                        op0=OP.mult, op1=OP.mult)
                    y1 = st.tile([128, 128], F32, name=f"vln_y1{it}")
                    nc.vector.scalar_tensor_tensor(
                        out=y1[:], in0=tn[:], scalar=1.5, in1=yv[:],
                        op0=OP.add, op1=OP.mult)
                    yv = y1
                # Vn = (Vraw - mu)*rv*vscale + vshift
                muv = mu[:].rearrange("p (c h) -> p c h", h=8)
                mub = bcast_ap(muv, [muv.ap[0], muv.ap[1], muv.ap[2], [0, 8]])
                rvv = yv[:].rearrange("p (c h) -> p c h", h=8)
                rvb = bcast_ap(rvv, [rvv.ap[0], rvv.ap[1], rvv.ap[2], [0, 8]])
                VC = st.tile([128, 1024], F32)
                vc4 = VC[:].rearrange("p (c h d) -> p c h d", h=8, d=8)
                nc.vector.tensor_tensor(out=vc4, in0=v4, in1=mub,
                                        op=OP.subtract)
                nc.vector.tensor_tensor(out=vc4, in0=vc4, in1=rvb, op=OP.mult)
                vsa = vscale
                vsb = bcast_ap(vsa, [vsa.ap[0], [0, 16], [1, 64]])
                vha = vshift
                vhb = bcast_ap(vha, [vha.ap[0], [0, 16], [1, 64]])
                vc3 = VC[:].rearrange("p (c hd) -> p c hd", hd=64)
                nc.vector.tensor_tensor(out=vc3, in0=vc3, in1=vsb, op=OP.mult)
                nc.vector.tensor_tensor(
                    out=Vn[:].rearrange("p (c hd) -> p c hd", hd=64),
                    in0=vc3, in1=vhb, op=OP.add)
                if use_beta:
                    for hf in range(2):
                        vsps = pps.tile([1, 512], F32, tag="vs")
                        nc.tensor.matmul(vsps[:], ones[:],
                                         Vn[:, hf * 512:(hf + 1) * 512],
                                         start=True, stop=True)
                        nc.vector.tensor_copy(
                            out=vs_sb[:, hf * 512:(hf + 1) * 512], in_=vsps[:])
                    vsv = vs_sb[:]
                    nc.vector.tensor_reduce(
                        out=Vsum[:],
                        in_=bcast_ap(vsv, [vsv.ap[0], [1, 64], [64, 16]]),
                        axis=AX, op=OP.add)

            # ---------------- main loop ----------------
            with tc.tile_pool(name="sps", bufs=2, space="PSUM") as sps, \
                 tc.tile_pool(name="aps", bufs=3, space="PSUM") as aps, \
                 tc.tile_pool(name="avps", bufs=1, space="PSUM") as avps, \
                 tc.tile_pool(name="le", bufs=2) as le, \
                 tc.tile_pool(name="lar", bufs=2) as lar, \
                 tc.tile_pool(name="lsq", bufs=2) as lsq, \
                 tc.tile_pool(name="lsm", bufs=2) as lsm, \
                 tc.tile_pool(name="lt", bufs=2) as lt, \
                 tc.tile_pool(name="lp", bufs=2) as lp, \
                 tc.tile_pool(name="lout", bufs=2) as lout:

                def front(t):
                    """QK + softmax-exp + denominator + tw for tile t."""
                    bd_t = BD[:, t * 128:(t + 1) * 128]
                    E = le.tile([128, N], BF16, tag="E", name=f"E{t}")
                    den4 = lsm.tile([128, 2], F32, tag="den4", name=f"d4_{t}")
                    for q2 in range(2):
                        s_t = sps.tile([128, 1024], F32, tag="s",
                                       name=f"s{t}_{q2}")
                        for qh in range(2):
                            j0 = (q2 * 2 + qh) * 512
                            nc.tensor.matmul(
                                s_t[:, qh * 512:(qh + 1) * 512], bd_t,
                                kT[:, j0:j0 + 512], start=True, stop=True)
                        nc.scalar.activation(
                            out=E[:, q2 * 1024:(q2 + 1) * 1024], in_=s_t[:],
                            func=AF.Exp, accum_out=den4[:, q2:q2 + 1])
                    den = lsm.tile([128, 1], F32, tag="den", name=f"dn{t}")
                    nc.vector.tensor_reduce(out=den[:], in_=den4[:], axis=AX,
                                            op=OP.add)
                    rden = lsm.tile([128, 1], F32, tag="rden", name=f"rd{t}")
                    nc.vector.reciprocal_approx_fast(out=rden[:], in_=den[:])
                    tw = lt.tile([128, 128], BF16, tag="tw", name=f"tw{t}")
                    nc.vector.tensor_scalar(out=tw[:], in0=WD,
                                            scalar1=rden[:, 0:1], scalar2=None,
                                            op0=OP.mult)
                    return E, tw

                E, tw = front(0)
                for t in range(NT):
                    Araw = lar.tile([128, N], BF16, tag="Araw")
                    # talk matmuls + PSUM evacuation (3 ACT copies, 1 DVE)
                    for b4 in range(4):
                        a_t = aps.tile([128, 512], F32, tag="a")
                        for cl in range(4):
                            c = b4 * 4 + cl
                            nc.tensor.matmul(a_t[:, cl * 128:(cl + 1) * 128],
                                             E[:, c * 128:(c + 1) * 128],
                                             tw[:], start=True, stop=True)
                        ar_b = Araw[:, b4 * 512:(b4 + 1) * 512]
                        if b4 < 3:
                            nc.scalar.copy(out=ar_b, in_=a_t[:])
                        else:
                            nc.vector.tensor_copy(out=ar_b, in_=a_t[:])
                    # emit deferred output stage of previous tile while the
                    # DVE would otherwise stall on the rsqrt chain
                    # squares + first tree level fused (custom DVE op):
                    # T1[j,(c,4g,i)] = A[g]^2 + A[g+4]^2
                    ar3 = Araw[:].rearrange("p (c k) -> p c k", k=128)
                    T1 = lsq.tile([128, 1024], BF16, tag="T1")
                    t1 = T1[:].rearrange("p (c k) -> p c k", k=64)
                    nc.vector._custom_dve(cust["sqsum2"], out=t1,
                                          in0=ar3[:, :, 0:64],
                                          in1=ar3[:, :, 64:128])
                    T2 = lsq.tile([128, 512], BF16, tag="T2")
                    t2 = T2[:].rearrange("p (c k) -> p c k", k=32)
                    nc.vector.tensor_tensor(out=t2, in0=t1[:, :, 0:32],
                                            in1=t1[:, :, 32:64], op=OP.add)
                    # vp = (T2a + eps) + T2b   [128, (c,i)=256] f32
                    VP = lsm.tile([128, 256], F32, tag="VP")
                    vpv = VP[:].rearrange("p (c i) -> p c i", i=16)
                    nc.vector.scalar_tensor_tensor(
                        out=vpv, in0=t2[:, :, 0:16], scalar=float(EPS),
                        in1=t2[:, :, 16:32], op0=OP.add, op1=OP.add)
                    # r = rsqrt(vp): magic + 1 Newton step
                    SH = lsm.tile([128, 256], I32, tag="SH")
                    nc.vector.tensor_scalar(out=SH[:], in0=VP.bitcast(I32)[:],
                                            scalar1=1, scalar2=None,
                                            op0=OP.logical_shift_right)
                    mg = magic[:]
                    mgb = bcast_ap(mg, [mg.ap[0], [0, 256]])
                    Y0 = lsm.tile([128, 256], F32, tag="Y0")
                    nc.gpsimd.tensor_tensor(out=Y0.bitcast(I32)[:], in0=mgb,
                                            in1=SH[:], op=OP.subtract)
                    # RB = y0*((y0^2*vp)*-0.5 + 1.5)  (fused Newton step)
                    RB = lsm.tile([128, 256], BF16, tag="RB")
                    nc.vector._custom_dve(cust["newton"], out=RB[:],
                                          in0=Y0[:], in1=VP[:],
                                          s0=-0.5, s1=1.5)
                    # P = Araw * r (broadcast over g), AV interleaved
                    P = lp.tile([128, N], BF16, tag="P")
                    rbv = RB[:].rearrange("p (c i) -> p c i", i=16)
                    av = avps.tile([128, 64], F32, tag="av")
                    for hf in range(2):
                        av_in = Araw[:, hf * 1024:(hf + 1) * 1024].rearrange(
                            "p (c g i) -> p c g i", g=8, i=16)
                        pv = P[:, hf * 1024:(hf + 1) * 1024].rearrange(
                            "p (c g i) -> p c g i", g=8, i=16)
                        rslice = rbv[:, hf * 8:(hf + 1) * 8, :]
                        rb = bcast_ap(rslice, [rslice.ap[0], rslice.ap[1],
                                               [0, 8], rslice.ap[2]])
                        nc.vector.tensor_mul(out=pv, in0=av_in, in1=rb)
                        for cl in range(8):
                            c = hf * 8 + cl
                            nc.tensor.matmul(av[:],
                                             P[:, c * 128:(c + 1) * 128],
                                             Vn[:, c * 64:(c + 1) * 64],
                                             start=(c == 0),
                                             stop=(c == 15 and not use_beta))
                    if use_beta:
                        nc.tensor.matmul(av[:], beta[:], Vsum[:],
                                         start=False, stop=True)
                    EXt = lout.tile([128, 64], F32, tag="EX")
                    nc.vector.tensor_tensor(out=EXt[:], in0=av[:],
                                            in1=mask, op=OP.mult)
                    RESt = lout.tile([128, 8], F32, tag="RES")
                    nc.vector.tensor_reduce(
                        out=RESt[:],
                        in_=EXt[:].rearrange("p (h d) -> p d h", h=8),
                        axis=AX, op=OP.add)
                    nc.sync.dma_start(
                        out=d_out[t * 128:(t + 1) * 128, :], in_=RESt[:])
                    if t + 1 < NT:
                        E, tw = front(t + 1)
    nc.compile()
    return nc


def _get_module(use_beta):
    if use_beta not in _CACHE:
        _CACHE[use_beta] = _build(use_beta)
    return _CACHE[use_beta]


def kernel(x, context, Wq, Wkv, g_v, b_v, W_talk, g_t, b_t, **_unused):
    from concourse.bass_utils import run_bass_kernel_spmd

    x = np.asarray(x, np.float32)
    context = np.asarray(context, np.float32)
    Wq = np.asarray(Wq, np.float32)
    Wkv = np.asarray(Wkv, np.float32)
    g_v = np.asarray(g_v, np.float32)
    b_v = np.asarray(b_v, np.float32)
    W_talk = np.asarray(W_talk, np.float32)
    g_t = np.asarray(g_t, np.float32)
    b_t = np.asarray(b_t, np.float32)

    use_beta = bool(np.any(b_t != 0.0))
    nc = _get_module(use_beta)

    wqT = (Wq.T * np.float32(DIM ** -0.5)).astype(np.float32)
    wkT = Wkv[:DIM, :].T
    wvT = Wkv[DIM:, :].T
    wp = np.concatenate([wqT, wkT, wvT], axis=1)

    S8 = np.float32(np.sqrt(8.0))
    Wc = (W_talk - W_talk.mean(axis=0, keepdims=True)) / S8
    WD = np.zeros((8, 16, 8, 16), np.float32)
    for i in range(16):
        WD[:, i, :, i] = Wc.T          # WD[h,i,g,i] = Wc[g,h]/sqrt8
    WD = WD.reshape(128, 128)
    maskf = np.zeros((8, 16, 8, 8), np.float32)
    for g in range(8):
        maskf[g, :, g, :] = 1.0
    maskf = maskf.reshape(128, 64)
    vscale = np.tile(g_v, 8) * np.repeat(g_t, 8) * S8      # [(h,d)]
    vshift = np.tile(b_v, 8) * np.repeat(g_t, 8) * S8
    cst = np.zeros((128, 320), np.float32)
    cst[:, 0:128] = WD
    cst[:, 128:192] = maskf
    cst[:, 192:256] = vscale[None, :]
    cst[:, 256:320] = vshift[None, :]

    import ml_dtypes
    BF = ml_dtypes.bfloat16
    wp16 = np.ascontiguousarray(wp).astype(BF)
    in_maps = []
    for c in range(NCORES):
        b = c // 4
        i0 = (c % 4) * ICHUNK
        im = {
            "xT": np.ascontiguousarray(x[b, i0:i0 + ICHUNK, :].T).astype(BF),
            "ctxT": np.ascontiguousarray(context[b].T).astype(BF),
            "wp": wp16, "cst": cst,
        }
        if use_beta:
            im["beta"] = np.ascontiguousarray(
                np.repeat(b_t, 16)[None, :] / S8).astype(BF)
            im["ones"] = np.ones((128, 1), np.float32).astype(BF)
        in_maps.append(im)
    trace_dir = globals().get("TRACE_TMPDIR")
    if trace_dir:
        res = run_bass_kernel_spmd(nc, in_maps, list(range(NCORES)),
                                   trace=True, tmpdir=trace_dir)
        globals()["LAST_EXEC_NS"] = res.exec_time_ns
    else:
        res = run_bass_kernel_spmd(nc, in_maps, list(range(NCORES)))
    out = np.empty((B, 2048, DIM), np.float32)
    for c in range(NCORES):
        b = c // 4
        i0 = (c % 4) * ICHUNK
        o = res.results[c]["out"].reshape(NT, 8, 16, 8)
        out[b, i0:i0 + ICHUNK, :] = (
            o.transpose(0, 2, 1, 3).reshape(ICHUNK, DIM))
    return out
